# revision 48
# baseline (speedup 1.0000x reference)
"""Multi-head attention (query-axis softmax variant) on 8 Trainium2 NeuronCores.

Problem: B=4, T=2048, C=1024, H=16, Dh=64.
  q/k/v = per-head projections of x; wei = (q k^T) * C**-0.5, causal-masked;
  softmax over the QUERY axis (axis=2 of (B,H,T,S)); out = attn @ v, concat
  heads, project with Wp and add bp.

Sharding: 8 cores = 4 batches x 2 head-groups (8 heads each).  Each core
computes a partial projection output for its batch; host sums the two
group partials per batch and adds the bias.

Per-core dataflow is fully "transposed" (features on partitions, tokens on
the free axis) so the query-axis softmax stats become free-axis reductions:
  xT (C, T) -> qT/kT per head-pair (128, T) -> scores W[s,t] per key-tile
  -> P = exp(W*scale) with the masked entries driven to 0 via a -1e30
  additive triangle, Z[s] = accumulated row sums from the Exp activation
  -> v' = v * (1/Z) -> attout^T[d,t] -> y = attout^T.T @ WpT.

The q/k projections run in fp8e4m3 DoubleRow mode (weights pre-scaled by
32, two 128-row c-tiles contracted per matmul at double pump); everything
else is bf16 with fp32 PSUM accumulation.  The causal -1e30 triangle is
added to the diagonal score blocks on the PE itself (I @ tri matmuls), Z
for single-block rows comes from a DVE free-axis reduce instead of the Act
accumulator, and y is returned as fp16 partials summed on the host.

The attention loop is software-pipelined: score matmuls (chunk-major so
the two heads' row groups pair up) run one iteration ahead of the Exp
pass, attout matmuls lag two iterations behind it, and the q/k
projections of the next pair, the v projection and most output-projection
groups are interleaved into the loop as PE filler steps.  attout psum
chunks live in pinned per-chunk slots (avc0..3) and are evacuated as
their causal accumulation completes; fillers alternate between a scores
pool slot and the freed avc banks so they never queue behind a
still-accumulating chunk.  Junk matmuls prepended to the fillers plus a
per-iteration wait-free junk pump into the freed avc0 bank keep the PE's
HAM clock-gate at full speed across dependency stalls (without them the
PE spends ~40% of the kernel at half clock and every schedule converges
to the same wall time).
"""
import numpy as np

T = 2048
C = 1024
H = 16
DH = 64
B = 4
SCALE = float(C) ** -0.5
NEG = -1e30
P = 128

_CACHE = {}


def _build_nc():
    import concourse.bacc as bacc
    import concourse.tile as tile
    import concourse.mybir as mybir
    from contextlib import ExitStack

    FP = mybir.dt.float32
    FR = mybir.dt.float32r
    BF = mybir.dt.bfloat16
    F8 = mybir.dt.float8e4
    DR = mybir.MatmulPerfMode.DoubleRow
    AX = mybir.AxisListType.X
    EXP = mybir.ActivationFunctionType.Exp

    nc = bacc.Bacc("TRN2", target_bir_lowering=False, debug=False, num_devices=8)

    xT_d = nc.declare_dram_parameter("xt", [C, T], BF, isOutput=False)
    x8_d = nc.declare_dram_parameter("xt8", [C, T], F8, isOutput=False)
    wq_d = nc.declare_dram_parameter("wq8", [C, 512], F8, isOutput=False)
    wk_d = nc.declare_dram_parameter("wk8", [C, 512], F8, isOutput=False)
    wv_d = nc.declare_dram_parameter("wv", [C, 512], BF, isOutput=False)
    wp_d = nc.declare_dram_parameter("wpt", [512, C], BF, isOutput=False)
    tri_d = nc.declare_dram_parameter("tri", [P, P], BF, isOutput=False)
    eye_d = nc.declare_dram_parameter("eye", [P, P], BF, isOutput=False)
    y_d = nc.declare_dram_parameter("y", [T, C], mybir.dt.float16, isOutput=True)

    NCT = C // P      # 8 c-tiles
    NST = T // P      # 16 s-tiles
    NTG = T // 512    # 4 t-groups

    with tile.TileContext(nc) as tc:
        with (
            tc.tile_pool(name="perm", bufs=1) as perm,
            tc.tile_pool(name="work", bufs=4) as work,
            tc.tile_pool(name="stat", bufs=3) as stat,
            tc.tile_pool(name="statv", bufs=4) as statv,
            tc.tile_pool(name="pao", bufs=1) as pao,
            tc.tile_pool(name="ps", bufs=2, space="PSUM") as pspool,
            tc.tile_pool(name="avps", bufs=4, space="PSUM") as avpool,
        ):
            tri = perm.tile([P, P], BF, tag="tri")
            nc.sync.dma_start(tri[:], tri_d[:])
            eye = perm.tile([P, P], BF, tag="eye")
            nc.sync.dma_start(eye[:], eye_d[:])
            v_sb = perm.tile([P, NST, 512], BF, tag="v")
            q_sb = perm.tile([P, 4, T], BF, tag="q")
            k_sb = perm.tile([P, 4, T], BF, tag="k")

            es = ExitStack()
            pxw = es.enter_context(tc.tile_pool(name="px", bufs=1))
            wpool = es.enter_context(tc.tile_pool(name="w", bufs=2))

            # DMA order matters: the single sync queue serializes transfers,
            # so land what phase 1 needs first (x8 + pair-0 qk weights),
            # then wv + xT (v projections start a few iterations in).
            x8 = pxw.tile([P, NCT // 2, 2, T], F8, tag="x8")
            nc.sync.dma_start(x8[:], x8_d.ap().rearrange("(a e c) t -> c a e t", c=P, e=2))
            w0 = {}
            for wd, tag in ((wq_d, "wq"), (wk_d, "wk")):
                wt = wpool.tile([P, NCT // 2, 2, P], F8, tag=tag)
                nc.sync.dma_start(
                    wt[:], wd.ap()[:, 0:P].rearrange("(a e c) m -> c a e m", c=P, e=2))
                w0[tag] = wt
            wv = pxw.tile([P, NCT, 512], BF, tag="wv")
            nc.sync.dma_start(wv[:], wv_d.ap().rearrange("(a c) m -> c a m", c=P))
            xT = pxw.tile([P, NCT, T], BF, tag="xT")
            nc.sync.dma_start(xT[:, :, :1024], xT_d.ap()[:, :1024].rearrange("(a c) t -> c a t", c=P))
            nc.sync.dma_start(xT[:, :, 1024:], xT_d.ap()[:, 1024:].rearrange("(a c) t -> c a t", c=P))

            # Warm up the PE's HAM clock gate while the big input DMAs land:
            # ~3us of continuous junk matmuls brings the PE to 2.4 GHz.
            warm = perm.tile([P, 512], BF, tag="warm")
            nc.vector.memset(warm[:], 0.0)
            for wi in range(14):
                wps = pspool.tile([P, 1024], FP, tag="ps")
                for _ in range(2):
                    nc.tensor.matmul(wps[:, :512], lhsT=warm[:, :P], rhs=warm[:],
                                     start=True, stop=True)

            def ldw_pump(n=3):
                """Psum-free PE activity: bare weight loads of the static
                warm tile.  Covers HAM clock-gate windows in phases where
                no psum bank is free for a junk matmul; the next real
                matmul's own weight load overwrites the PE registers."""
                for _ in range(n):
                    nc.tensor.ldweights(warm[:, :P])

            # ---- phase 1: qT/kT per pair (128 = [h0 d, h1 d], T), bf16 out ----
            def emit_qk_steps(p, wts=None):
                """Returns a list of closures; each emits one 512-col psum group.

                fp8e4 DoubleRow: each matmul contracts two 128-row c-tiles at
                once (lhsT [128, 2, M], rhs [128, 2, N]) at double pump rate.
                """
                if wts is not None:
                    wqt, wkt = wts
                else:
                    wqt = wpool.tile([P, NCT // 2, 2, P], F8, tag="wq")
                    wkt = wpool.tile([P, NCT // 2, 2, P], F8, tag="wk")
                    nc.sync.dma_start(
                        wqt[:], wq_d.ap()[:, P * p:P * p + P].rearrange("(a e c) m -> c a e m", c=P, e=2))
                    nc.sync.dma_start(
                        wkt[:], wk_d.ap()[:, P * p:P * p + P].rearrange("(a e c) m -> c a e m", c=P, e=2))
                steps = []
                for wt, dst in ((wqt, q_sb), (wkt, k_sb)):
                    for g in range(NTG):
                        def step(wt=wt, dst=dst, g=g, p=p, slot=None):
                            psq = fill_ps(slot, f"qkps{p}_{g}_{dst is k_sb}")
                            nc.tensor.matmul(psq[:, :512], lhsT=warm[:, :P],
                                              rhs=warm[:], start=True, stop=True)
                            for a2 in range(NCT // 2):
                                nc.tensor.matmul(
                                    psq[:, :512], lhsT=wt[:, a2],
                                    rhs=x8[:, a2, :, 512 * g:512 * g + 512],
                                    start=(a2 == 0), stop=(a2 == NCT // 2 - 1),
                                    perf_mode=DR)
                            nc.vector.tensor_copy(dst[:, p, 512 * g:512 * g + 512], psq[:, :512])
                        steps.append(step)
                return steps

            def fill_ps(slot, name):
                """Psum for filler steps: either a scores-pool slot (short
                wait: one exp block behind) or a pinned freed avc slot."""
                if slot is None:
                    return pspool.tile([P, 1024], FP, tag="ps", name=name)
                return avpool.tile([P, 512], FP, tag=slot, bufs=1, name=name)

            def v_step(st, slot=None):
                ps = fill_ps(slot, f"vps{st}")
                for ct in range(NCT):
                    nc.tensor.matmul(
                        ps[:, :512],
                        lhsT=xT[:, ct, P * st:P * st + P],
                        rhs=wv[:, ct, :],
                        start=(ct == 0), stop=(ct == NCT - 1))
                nc.vector.tensor_copy(v_sb[:, st, :], ps[:, :512])

            # ---- serial prefix: pair-0 q (all groups) + k groups 0-1; the
            # first scores only need q fully and k's low t-groups, so k2/k3
            # are deferred into the loop as early fills.
            qk0 = emit_qk_steps(0, wts=(w0["wq"], w0["wk"]))
            for step in qk0[:6]:
                step()
            qk0_rest = qk0[6:]
            ldw_pump(6)

            # ---- phase 2: attention; the two heads of a pair run in lockstep
            # (score matmuls in different PE row groups, attout matmuls in
            # different column groups), and the attout matmuls of iteration
            # i-1 are emitted after the score matmuls of iteration i so the
            # in-order PE queue never stalls on the Exp results.
            ao = pao.tile([P, 4, T], BF, tag="ao")

            def emit_scores(p, i):
                """PE: both heads' score matmuls (chunk-interleaved so the
                row-group pair runs concurrently); DVE: diag masks."""
                t0 = P * i
                blocks = [(t0, 1024), (1024, 2048)] if i < 8 else [(t0, 2048)]
                prows = [work.tile([P, T], BF, tag="prow", bufs=8,
                                   name=f"prow{p}_{i}_{h}") for h in range(2)]
                zps = [stat.tile([P, 2], FP, tag="zp", bufs=8,
                                 name=f"zp{p}_{i}_{h}") for h in range(2)]
                tiles = []
                for bi, (lo, hi) in enumerate(blocks):
                    sps2 = [pspool.tile([P, 1024], FP, tag="ps",
                                        name=f"sps{p}_{i}_{bi}_{h}") for h in range(2)]
                    # chunk-major emission so adjacent queue entries target
                    # the two different PE row groups and actually pair up
                    for clo in range(lo, hi, 512):
                        for hl in range(2):
                            hb = 64 * hl
                            chi = min(clo + 512, hi)
                            diag = lo == t0 and clo == lo
                            nc.tensor.matmul(
                                sps2[hl][:, clo - lo:chi - lo],
                                lhsT=k_sb[hb:hb + 64, p, t0:t0 + P],
                                rhs=q_sb[hb:hb + 64, p, clo:chi],
                                start=True, stop=not diag)
                    if lo == t0:
                        # causal mask: add the -1e30 lower triangle to the
                        # diagonal blocks on the PE (I @ tri).  Emitted after
                        # BOTH heads' score chains so the full-width eye
                        # matmuls don't break the row-group pairing.
                        for hl in range(2):
                            nc.tensor.matmul(
                                sps2[hl][:, 0:P], lhsT=eye[:], rhs=tri[:],
                                start=False, stop=True, skip_group_check=True)
                    tiles.append((sps2, bi, lo, hi))
                return dict(i=i, t0=t0, nb=len(blocks), prows=prows, zps=zps, tiles=tiles)

            def emit_exps(sc):
                for (sps2, bi, lo, hi) in sc["tiles"]:
                    for hl in range(2):
                        # single-block rows skip the Act accumulator (and its
                        # costly READ_ACCUMULATOR); Z comes from a DVE reduce.
                        acc = sc["zps"][hl][:, bi:bi + 1] if sc["nb"] == 2 else None
                        nc.scalar.activation(
                            sc["prows"][hl][:, lo:hi], sps2[hl][:, :hi - lo], EXP,
                            scale=SCALE / 1024.0, accum_out=acc)

            def emit_stats(p, sc):
                vps = []
                for hl in range(2):
                    z = stat.tile([P, 1], FP, tag="z", name=f"z{p}_{sc['i']}_{hl}")
                    if sc["nb"] == 2:
                        nc.gpsimd.tensor_add(z[:], sc["zps"][hl][:, 0:1], sc["zps"][hl][:, 1:2])
                    else:
                        nc.vector.tensor_reduce(
                            z[:], sc["prows"][hl][:, sc["t0"]:T],
                            mybir.AxisListType.X, mybir.AluOpType.add)
                    rz = stat.tile([P, 1], FP, tag="rz", name=f"rz{p}_{sc['i']}_{hl}")
                    nc.vector.reciprocal(rz[:], z[:])
                    vp = statv.tile([P, 64], BF, tag="vp", bufs=6, name=f"vp{p}_{sc['i']}_{hl}")
                    hh = 64 * (2 * p + hl)
                    nc.vector.tensor_scalar_mul(vp[:], v_sb[:, sc["i"], hh:hh + 64], rz[:])
                    vps.append(vp)
                return vps

            wpt = pao.tile([P, 4, C], BF, tag="wpt")
            nc.sync.dma_start(wpt[:], wp_d.ap().rearrange("(a c) m -> c a m", c=P))

            def proj_group(tt, nb, junk=True, slot="avc0"):
                ps = fill_ps(slot, f"pps{tt}_{nb}")
                if junk:
                    nc.tensor.matmul(ps[:, :512], lhsT=warm[:, :P], rhs=warm[:],
                                     start=True, stop=True)
                for pp in range(4):
                    nc.tensor.matmul(
                        ps[:, :512], lhsT=ao[:, pp, P * tt:P * tt + P],
                        rhs=wpt[:, pp, 512 * nb:512 * nb + 512],
                        start=(pp == 0), stop=(pp == 3))
                yt = work.tile([P, 512], mybir.dt.float16, tag="yt", bufs=2,
                               name=f"yt{tt}_{nb}")
                nc.vector.tensor_copy(yt[:], ps[:, :512])
                nc.sync.dma_start(y_d.ap()[P * tt:P * tt + P, 512 * nb:512 * nb + 512], yt[:])

            for p in range(4):
                if p == 0:
                    fill = qk0_rest + [
                        (lambda st=st: (lambda slot=None: v_step(st, slot=slot)))()
                        for st in range(8, NST)] + emit_qk_steps(1)
                elif p < 3:
                    fill = emit_qk_steps(p + 1)
                else:
                    fill = [(lambda tt=tt, nb=nb: (lambda slot=None: proj_group(tt, nb, slot=slot)))()
                            for tt in range(12) for nb in range(2)]
                avc = [avpool.tile([P, 512], FP, tag=f"avc{c}", bufs=1,
                                   name=f"avc{p}_{c}") for c in range(NTG)]
                pend = []
                done_av = -1
                evacd = 0
                nfill = 0
                sc = emit_scores(p, 0)
                for i in range(NST):
                    sc_next = emit_scores(p, i + 1) if i < NST - 1 else None
                    emit_exps(sc)
                    if i < 6:
                        ldw_pump(3)
                    if len(pend) >= 2 or (i == NST - 1 and pend):
                        pend.pop(0)()
                        done_av += 1
                    # evacuate finished attout chunks (frees their psum bank
                    # for the qk / projection filler steps)
                    if evacd < 3 and done_av == 4 * evacd + 3:
                        nc.vector.tensor_copy(
                            ao[:, p, 512 * evacd:512 * evacd + 512], avc[evacd][:])
                        evacd += 1
                    if p == 0 and i < 4:
                        v_step(2 * i)
                        v_step(2 * i + 1)
                    # fills alternate between a scores-pool slot (short wait)
                    # and the freed avc banks, so no fill chain queues behind
                    # an attout chunk that is still accumulating.
                    if fill and i >= 5:
                        n = 2 if len(fill) > 2 * (NST - 1 - i) else 1
                        for _ in range(min(n, len(fill))):
                            slots = [None, "avc0", None, "avc1"]
                            slot = slots[nfill % 4] if evacd >= 2 else \
                                (None if nfill % 2 == 0 else ("avc0" if evacd >= 1 else None))
                            fill.pop(0)(slot=slot)
                            nfill += 1
                    if evacd >= 1:
                        # wait-free junk pump into an already-freed attout
                        # bank: keeps the PE's HAM clock-gate at full speed
                        # across the dependency stalls of this iteration.
                        jp = avpool.tile([P, 512], FP, tag="avc0", bufs=1,
                                         name=f"jp{p}_{i}")
                        nc.tensor.matmul(jp[:, :512], lhsT=warm[:, :P],
                                         rhs=warm[:], start=True, stop=True)
                    else:
                        # early iterations have no free psum bank; use bare
                        # weight loads to keep the PE array active instead.
                        ldw_pump(4)
                    vps = emit_stats(p, sc)

                    def make_av(i=i, t0=P * i, vps=vps, prows=sc["prows"]):
                        def emit():
                            for c in range(NTG):
                                clo, chi = 512 * c, 512 * c + 512
                                lo2 = max(clo, t0)
                                if lo2 >= chi:
                                    continue
                                for hl in range(2):
                                    hb = 64 * hl
                                    nc.tensor.matmul(
                                        avc[c][hb:hb + 64, lo2 - clo:512],
                                        lhsT=vps[hl][:], rhs=prows[hl][:, lo2:chi],
                                        start=(i == 0), stop=(i == 4 * c + 3))
                        return emit

                    pend.append(make_av())
                    sc = sc_next
                for ei, e in enumerate(pend):
                    e()
                    jb = avpool.tile([P, 512], FP, tag="avc0", bufs=1,
                                     name=f"jb{p}_{ei}")
                    nc.tensor.matmul(jb[:, :512], lhsT=warm[:, :P], rhs=warm[:],
                                     start=True, stop=True)
                nc.vector.tensor_copy(ao[:, p, 512 * 3:], avc[3][:])
                for fi, step in enumerate(fill):
                    step(slot=["avc0", "avc1", None, "avc2"][fi % 4])
            es.close()

            # ---- phase 3: projection y[t, c'] ----  (avc3 is free here: a
            # dedicated wait-free junk chain on it keeps the clock up while
            # the serial projection groups wait on their psum slots)
            for k, (tt, nb) in enumerate(
                    (tt, nb) for tt in range(12, NST) for nb in range(2)):
                proj_group(tt, nb, junk=False,
                           slot=["avc0", "avc1", None, "avc2"][k % 4])
                jt = avpool.tile([P, 512], FP, tag="avc3", bufs=1,
                                 name=f"jt{k}")
                nc.tensor.matmul(jt[:, :512], lhsT=warm[:, :P], rhs=warm[:],
                                 start=True, stop=True)

    nc.compile()
    return nc


def _get_nc():
    if "nc" not in _CACHE:
        _CACHE["nc"] = _build_nc()
    return _CACHE["nc"]


def _in_maps(x, Wq, Wk, Wv, Wp):
    import ml_dtypes

    def f8(a):
        return np.clip(a, -240.0, 240.0).astype(ml_dtypes.float8_e4m3)

    tri = np.tril(np.full((P, P), NEG, np.float32), -1).astype(ml_dtypes.bfloat16)
    eye = np.eye(P, dtype=np.float32).astype(ml_dtypes.bfloat16)
    maps = []
    for b in range(B):
        xT = np.ascontiguousarray(x[b].T)
        for g in range(2):
            heads = range(8 * g, 8 * g + 8)
            # q/k projection weights are pre-scaled by 32 so they sit in the
            # fp8e4m3 sweet spot (std ~1); the exp scale divides the extra
            # 32*32 back out.
            wq = np.ascontiguousarray(np.concatenate([Wq[h] for h in heads], 1))
            wk = np.ascontiguousarray(np.concatenate([Wk[h] for h in heads], 1))
            maps.append({
                "xt": xT.astype(ml_dtypes.bfloat16),
                "xt8": f8(xT),
                "wq8": f8(32.0 * wq),
                "wk8": f8(32.0 * wk),
                "wv": np.ascontiguousarray(np.concatenate([Wv[h] for h in heads], 1)).astype(ml_dtypes.bfloat16),
                "wpt": np.ascontiguousarray(Wp[:, 512 * g:512 * g + 512].T).astype(ml_dtypes.bfloat16),
                "tri": tri,
                "eye": eye,
            })
    return maps


def kernel(x, Wq, Wk, Wv, Wp, bp):
    from concourse.bass_utils import run_bass_kernel_spmd

    x = np.asarray(x, np.float32)
    Wq = np.asarray(Wq, np.float32)
    Wk = np.asarray(Wk, np.float32)
    Wv = np.asarray(Wv, np.float32)
    Wp = np.asarray(Wp, np.float32)
    bp = np.asarray(bp, np.float32)

    nc = _get_nc()
    res = run_bass_kernel_spmd(nc, _in_maps(x, Wq, Wk, Wv, Wp), list(range(8)))
    y = np.empty((B, T, C), np.float32)
    for b in range(B):
        y[b] = (res.results[2 * b]["y"].astype(np.float32)
                + res.results[2 * b + 1]["y"].astype(np.float32) + bp)
    return y



# revision 51
# speedup vs baseline: 1.0949x; 1.0949x over previous
"""Multi-head attention (query-axis softmax variant) on 8 Trainium2 NeuronCores.

Problem: B=4, T=2048, C=1024, H=16, Dh=64.
  q/k/v = per-head projections of x; wei = (q k^T) * C**-0.5, causal-masked;
  softmax over the QUERY axis (axis=2 of (B,H,T,S)); out = attn @ v, concat
  heads, project with Wp and add bp.

Sharding: 8 cores = 4 batches x 2 head-groups (8 heads each).  Each core
computes a partial projection output for its batch; host sums the two
group partials per batch and adds the bias.

Per-core dataflow is fully "transposed" (features on partitions, tokens on
the free axis) so the query-axis softmax stats become free-axis reductions:
  xT (C, T) -> qT/kT per head-pair (128, T) -> scores W[s,t] per key-tile
  -> P = exp(W*scale) with the masked entries driven to 0 via a -1e30
  additive triangle, Z[s] = accumulated row sums from the Exp activation
  -> v' = v * (1/Z) -> attout^T[d,t] -> y = attout^T.T @ WpT.

The q/k projections run in fp8e4m3 DoubleRow mode (weights pre-scaled by
32, two 128-row c-tiles contracted per matmul at double pump); everything
else is bf16 with fp32 PSUM accumulation.  The causal -1e30 triangle is
added to the diagonal score blocks on the PE itself (I @ tri matmuls), Z
for single-block rows comes from a DVE free-axis reduce instead of the Act
accumulator, and y is returned as fp16 partials summed on the host.

The attention loop is software-pipelined: score matmuls (chunk-major so
the two heads' row groups pair up) run one iteration ahead of the Exp
pass, attout matmuls lag two iterations behind it, and the q/k
projections of the next pair, the v projection and most output-projection
groups are interleaved into the loop as PE filler steps.  attout psum
chunks live in pinned per-chunk slots (avc0..3) and are evacuated as
their causal accumulation completes; fillers alternate between a scores
pool slot and the freed avc banks so they never queue behind a
still-accumulating chunk.  Junk matmuls prepended to the fillers plus a
per-iteration wait-free junk pump into the freed avc0 bank keep the PE's
HAM clock-gate at full speed across dependency stalls (without them the
PE spends ~40% of the kernel at half clock and every schedule converges
to the same wall time).
"""
import numpy as np

T = 2048
C = 1024
H = 16
DH = 64
B = 4
SCALE = float(C) ** -0.5
NEG = -1e30
P = 128

_CACHE = {}


def _build_nc():
    import concourse.bacc as bacc
    import concourse.tile as tile
    import concourse.mybir as mybir
    from contextlib import ExitStack

    FP = mybir.dt.float32
    FR = mybir.dt.float32r
    BF = mybir.dt.bfloat16
    F8 = mybir.dt.float8e4
    DR = mybir.MatmulPerfMode.DoubleRow
    AX = mybir.AxisListType.X
    EXP = mybir.ActivationFunctionType.Exp

    nc = bacc.Bacc("TRN2", target_bir_lowering=False, debug=False, num_devices=8)

    xT_d = nc.declare_dram_parameter("xt", [C, T], BF, isOutput=False)
    x8_d = nc.declare_dram_parameter("xt8", [C, T], F8, isOutput=False)
    wq_d = nc.declare_dram_parameter("wq8", [C, 512], F8, isOutput=False)
    wk_d = nc.declare_dram_parameter("wk8", [C, 512], F8, isOutput=False)
    wv_d = nc.declare_dram_parameter("wv", [C, 512], BF, isOutput=False)
    wp_d = nc.declare_dram_parameter("wpt", [512, C], BF, isOutput=False)
    tri_d = nc.declare_dram_parameter("tri", [P, P], BF, isOutput=False)
    eye_d = nc.declare_dram_parameter("eye", [P, P], BF, isOutput=False)
    y_d = nc.declare_dram_parameter("y", [T, C], mybir.dt.float16, isOutput=True)

    NCT = C // P      # 8 c-tiles
    NST = T // P      # 16 s-tiles
    NTG = T // 512    # 4 t-groups

    with tile.TileContext(nc) as tc:
        with (
            tc.tile_pool(name="perm", bufs=1) as perm,
            tc.tile_pool(name="work", bufs=4) as work,
            tc.tile_pool(name="stat", bufs=3) as stat,
            tc.tile_pool(name="statv", bufs=4) as statv,
            tc.tile_pool(name="pao", bufs=1) as pao,
            tc.tile_pool(name="ps", bufs=2, space="PSUM") as pspool,
            tc.tile_pool(name="avps", bufs=4, space="PSUM") as avpool,
        ):
            tri = perm.tile([P, P], BF, tag="tri")
            nc.sync.dma_start(tri[:], tri_d[:])
            eye = perm.tile([P, P], BF, tag="eye")
            nc.sync.dma_start(eye[:], eye_d[:])
            v_sb = perm.tile([P, NST, 512], BF, tag="v")
            q_sb = perm.tile([P, 4, T], BF, tag="q")
            k_sb = perm.tile([P, 4, T], BF, tag="k")

            es = ExitStack()
            pxw = es.enter_context(tc.tile_pool(name="px", bufs=1))
            wpool = es.enter_context(tc.tile_pool(name="w", bufs=2))

            # DMA order matters: the single sync queue serializes transfers,
            # so land what phase 1 needs first (x8 + pair-0 qk weights),
            # then wv + xT (v projections start a few iterations in).
            x8 = pxw.tile([P, NCT // 2, 2, T], F8, tag="x8")
            nc.sync.dma_start(x8[:], x8_d.ap().rearrange("(a e c) t -> c a e t", c=P, e=2))
            w0 = {}
            for wd, tag in ((wq_d, "wq"), (wk_d, "wk")):
                wt = wpool.tile([P, NCT // 2, 2, P], F8, tag=tag)
                nc.sync.dma_start(
                    wt[:], wd.ap()[:, 0:P].rearrange("(a e c) m -> c a e m", c=P, e=2))
                w0[tag] = wt
            wv = pxw.tile([P, NCT, 512], BF, tag="wv")
            nc.sync.dma_start(wv[:], wv_d.ap().rearrange("(a c) m -> c a m", c=P))
            xT = pxw.tile([P, NCT, T], BF, tag="xT")
            nc.sync.dma_start(xT[:, :, :1024], xT_d.ap()[:, :1024].rearrange("(a c) t -> c a t", c=P))
            nc.sync.dma_start(xT[:, :, 1024:], xT_d.ap()[:, 1024:].rearrange("(a c) t -> c a t", c=P))

            # Warm up the PE's HAM clock gate while the big input DMAs land:
            # ~3us of continuous junk matmuls brings the PE to 2.4 GHz.
            warm = perm.tile([P, 512], BF, tag="warm")
            nc.vector.memset(warm[:], 0.0)
            for wi in range(14):
                wps = pspool.tile([P, 1024], FP, tag="ps")
                for _ in range(2):
                    nc.tensor.matmul(wps[:, :512], lhsT=warm[:, :P], rhs=warm[:],
                                     start=True, stop=True)

            def ldw_pump(n=3):
                """Psum-free PE activity: bare weight loads of the static
                warm tile.  Covers HAM clock-gate windows in phases where
                no psum bank is free for a junk matmul; the next real
                matmul's own weight load overwrites the PE registers."""
                for _ in range(n):
                    nc.tensor.ldweights(warm[:, :P])

            # ---- phase 1: qT/kT per pair (128 = [h0 d, h1 d], T), bf16 out ----
            def emit_qk_steps(p, wts=None):
                """Returns a list of closures; each emits one 512-col psum group.

                fp8e4 DoubleRow: each matmul contracts two 128-row c-tiles at
                once (lhsT [128, 2, M], rhs [128, 2, N]) at double pump rate.
                """
                if wts is not None:
                    wqt, wkt = wts
                else:
                    wqt = wpool.tile([P, NCT // 2, 2, P], F8, tag="wq")
                    wkt = wpool.tile([P, NCT // 2, 2, P], F8, tag="wk")
                    nc.sync.dma_start(
                        wqt[:], wq_d.ap()[:, P * p:P * p + P].rearrange("(a e c) m -> c a e m", c=P, e=2))
                    nc.sync.dma_start(
                        wkt[:], wk_d.ap()[:, P * p:P * p + P].rearrange("(a e c) m -> c a e m", c=P, e=2))
                steps = []
                for wt, dst in ((wqt, q_sb), (wkt, k_sb)):
                    for g in range(NTG):
                        def step(wt=wt, dst=dst, g=g, p=p, slot=None):
                            psq = fill_ps(slot, f"qkps{p}_{g}_{dst is k_sb}")
                            nc.tensor.matmul(psq[:, :512], lhsT=warm[:, :P],
                                              rhs=warm[:], start=True, stop=True)
                            for a2 in range(NCT // 2):
                                nc.tensor.matmul(
                                    psq[:, :512], lhsT=wt[:, a2],
                                    rhs=x8[:, a2, :, 512 * g:512 * g + 512],
                                    start=(a2 == 0), stop=(a2 == NCT // 2 - 1),
                                    perf_mode=DR)
                            nc.vector.tensor_copy(dst[:, p, 512 * g:512 * g + 512], psq[:, :512])
                        steps.append(step)
                return steps

            def fill_ps(slot, name):
                """Psum for filler steps: either a scores-pool slot (short
                wait: one exp block behind) or a pinned freed avc slot."""
                if slot is None:
                    return pspool.tile([P, 1024], FP, tag="ps", name=name)
                return avpool.tile([P, 512], FP, tag=slot, bufs=1, name=name)

            def v_step(st, slot=None):
                ps = fill_ps(slot, f"vps{st}")
                for ct in range(NCT):
                    nc.tensor.matmul(
                        ps[:, :512],
                        lhsT=xT[:, ct, P * st:P * st + P],
                        rhs=wv[:, ct, :],
                        start=(ct == 0), stop=(ct == NCT - 1))
                nc.vector.tensor_copy(v_sb[:, st, :], ps[:, :512])

            # ---- serial prefix: pair-0 q (all groups) + k groups 0-1; the
            # first scores only need q fully and k's low t-groups, so k2/k3
            # are deferred into the loop as early fills.
            qk0 = emit_qk_steps(0, wts=(w0["wq"], w0["wk"]))
            for step in qk0[:6]:
                step()
            qk0_rest = qk0[6:]

            # ---- phase 2: attention; the two heads of a pair run in lockstep
            # (score matmuls in different PE row groups, attout matmuls in
            # different column groups), and the attout matmuls of iteration
            # i-1 are emitted after the score matmuls of iteration i so the
            # in-order PE queue never stalls on the Exp results.
            ao = pao.tile([P, 4, T], BF, tag="ao")

            def emit_scores(p, i):
                """PE: both heads' score matmuls (chunk-interleaved so the
                row-group pair runs concurrently); DVE: diag masks."""
                t0 = P * i
                blocks = [(t0, 1024), (1024, 2048)] if i < 8 else [(t0, 2048)]
                prows = [work.tile([P, T], BF, tag="prow", bufs=8,
                                   name=f"prow{p}_{i}_{h}") for h in range(2)]
                zps = [stat.tile([P, 2], FP, tag="zp", bufs=8,
                                 name=f"zp{p}_{i}_{h}") for h in range(2)]
                tiles = []
                for bi, (lo, hi) in enumerate(blocks):
                    sps2 = [pspool.tile([P, 1024], FP, tag="ps",
                                        name=f"sps{p}_{i}_{bi}_{h}") for h in range(2)]
                    # chunk-major emission so adjacent queue entries target
                    # the two different PE row groups and actually pair up
                    for clo in range(lo, hi, 512):
                        for hl in range(2):
                            hb = 64 * hl
                            chi = min(clo + 512, hi)
                            diag = lo == t0 and clo == lo
                            nc.tensor.matmul(
                                sps2[hl][:, clo - lo:chi - lo],
                                lhsT=k_sb[hb:hb + 64, p, t0:t0 + P],
                                rhs=q_sb[hb:hb + 64, p, clo:chi],
                                start=True, stop=not diag)
                    if lo == t0:
                        # causal mask: add the -1e30 lower triangle to the
                        # diagonal blocks on the PE (I @ tri).  Emitted after
                        # BOTH heads' score chains so the full-width eye
                        # matmuls don't break the row-group pairing.
                        for hl in range(2):
                            nc.tensor.matmul(
                                sps2[hl][:, 0:P], lhsT=eye[:], rhs=tri[:],
                                start=False, stop=True, skip_group_check=True)
                    tiles.append((sps2, bi, lo, hi))
                return dict(i=i, t0=t0, nb=len(blocks), prows=prows, zps=zps, tiles=tiles)

            def emit_exps(sc):
                for (sps2, bi, lo, hi) in sc["tiles"]:
                    for hl in range(2):
                        # only the big second block keeps the Act accumulator;
                        # every other Z partial comes from a DVE free-axis
                        # reduce, keeping costly READ_ACCUMULATORs off the Act
                        # queue's critical path.
                        acc = (sc["zps"][hl][:, bi:bi + 1]
                               if sc["nb"] == 2 and bi == 1 else None)
                        nc.scalar.activation(
                            sc["prows"][hl][:, lo:hi], sps2[hl][:, :hi - lo], EXP,
                            scale=SCALE / 1024.0, accum_out=acc)

            def emit_stats(p, sc):
                vps = []
                for hl in range(2):
                    z = stat.tile([P, 1], FP, tag="z", name=f"z{p}_{sc['i']}_{hl}")
                    if sc["nb"] == 2:
                        zr = stat.tile([P, 1], FP, tag="zr",
                                       name=f"zr{p}_{sc['i']}_{hl}")
                        nc.vector.tensor_reduce(
                            zr[:], sc["prows"][hl][:, sc["t0"]:1024],
                            mybir.AxisListType.X, mybir.AluOpType.add)
                        nc.vector.tensor_add(z[:], zr[:], sc["zps"][hl][:, 1:2])
                    else:
                        nc.vector.tensor_reduce(
                            z[:], sc["prows"][hl][:, sc["t0"]:T],
                            mybir.AxisListType.X, mybir.AluOpType.add)
                    rz = stat.tile([P, 1], FP, tag="rz", name=f"rz{p}_{sc['i']}_{hl}")
                    nc.vector.reciprocal(rz[:], z[:])
                    vp = statv.tile([P, 64], BF, tag="vp", bufs=6, name=f"vp{p}_{sc['i']}_{hl}")
                    hh = 64 * (2 * p + hl)
                    nc.vector.tensor_scalar_mul(vp[:], v_sb[:, sc["i"], hh:hh + 64], rz[:])
                    vps.append(vp)
                return vps

            wpt = pao.tile([P, 4, C], BF, tag="wpt")
            nc.sync.dma_start(wpt[:], wp_d.ap().rearrange("(a c) m -> c a m", c=P))

            def proj_group(tt, nb, junk=True, slot="avc0"):
                ps = fill_ps(slot, f"pps{tt}_{nb}")
                if junk:
                    nc.tensor.matmul(ps[:, :512], lhsT=warm[:, :P], rhs=warm[:],
                                     start=True, stop=True)
                for pp in range(4):
                    nc.tensor.matmul(
                        ps[:, :512], lhsT=ao[:, pp, P * tt:P * tt + P],
                        rhs=wpt[:, pp, 512 * nb:512 * nb + 512],
                        start=(pp == 0), stop=(pp == 3))
                yt = work.tile([P, 512], mybir.dt.float16, tag="yt", bufs=2,
                               name=f"yt{tt}_{nb}")
                nc.vector.tensor_copy(yt[:], ps[:, :512])
                nc.sync.dma_start(y_d.ap()[P * tt:P * tt + P, 512 * nb:512 * nb + 512], yt[:])

            for p in range(4):
                if p == 0:
                    fill = qk0_rest + [
                        (lambda st=st: (lambda slot=None: v_step(st, slot=slot)))()
                        for st in range(8, NST)] + emit_qk_steps(1)
                elif p < 3:
                    fill = emit_qk_steps(p + 1)
                else:
                    fill = [(lambda tt=tt, nb=nb: (lambda slot=None: proj_group(tt, nb, slot=slot)))()
                            for tt in range(12) for nb in range(2)]
                avc = [avpool.tile([P, 512], FP, tag=f"avc{c}", bufs=1,
                                   name=f"avc{p}_{c}") for c in range(NTG)]
                pend = []
                done_av = -1
                evacd = 0
                nfill = 0
                sc = emit_scores(p, 0)
                for i in range(NST):
                    sc_next = emit_scores(p, i + 1) if i < NST - 1 else None
                    emit_exps(sc)
                    if len(pend) >= 2 or (i == NST - 1 and pend):
                        pend.pop(0)()
                        done_av += 1
                    # evacuate finished attout chunks (frees their psum bank
                    # for the qk / projection filler steps)
                    if evacd < 3 and done_av == 4 * evacd + 3:
                        nc.vector.tensor_copy(
                            ao[:, p, 512 * evacd:512 * evacd + 512], avc[evacd][:])
                        evacd += 1
                    if p == 0 and i < 4:
                        v_step(2 * i)
                        v_step(2 * i + 1)
                    # fills alternate between a scores-pool slot (short wait)
                    # and the freed avc banks, so no fill chain queues behind
                    # an attout chunk that is still accumulating.
                    if fill and i >= 5:
                        n = 2 if len(fill) > 2 * (NST - 1 - i) else 1
                        for _ in range(min(n, len(fill))):
                            slots = [None, "avc0", None, "avc1"]
                            slot = slots[nfill % 4] if evacd >= 2 else \
                                (None if nfill % 2 == 0 else ("avc0" if evacd >= 1 else None))
                            fill.pop(0)(slot=slot)
                            nfill += 1
                    if evacd >= 1:
                        # wait-free junk pump into an already-freed attout
                        # bank: keeps the PE's HAM clock-gate at full speed
                        # across the dependency stalls of this iteration.
                        jp = avpool.tile([P, 512], FP, tag="avc0", bufs=1,
                                         name=f"jp{p}_{i}")
                        nc.tensor.matmul(jp[:, :512], lhsT=warm[:, :P],
                                         rhs=warm[:], start=True, stop=True)
                    else:
                        # early iterations have no free psum bank; use bare
                        # weight loads to keep the PE array active instead.
                        ldw_pump(4)
                    vps = emit_stats(p, sc)

                    def make_av(i=i, t0=P * i, vps=vps, prows=sc["prows"]):
                        def emit():
                            for c in range(NTG):
                                clo, chi = 512 * c, 512 * c + 512
                                lo2 = max(clo, t0)
                                if lo2 >= chi:
                                    continue
                                for hl in range(2):
                                    hb = 64 * hl
                                    nc.tensor.matmul(
                                        avc[c][hb:hb + 64, lo2 - clo:512],
                                        lhsT=vps[hl][:], rhs=prows[hl][:, lo2:chi],
                                        start=(i == 0), stop=(i == 4 * c + 3))
                        return emit

                    pend.append(make_av())
                    sc = sc_next
                for ei, e in enumerate(pend):
                    e()
                    jb = avpool.tile([P, 512], FP, tag="avc0", bufs=1,
                                     name=f"jb{p}_{ei}")
                    nc.tensor.matmul(jb[:, :512], lhsT=warm[:, :P], rhs=warm[:],
                                     start=True, stop=True)
                for fi, step in enumerate(fill):
                    step(slot=["avc0", "avc1", None, "avc2"][fi % 4])
                nc.vector.tensor_copy(ao[:, p, 512 * 3:], avc[3][:])
            es.close()

            # ---- phase 3: projection y[t, c'] ----  (avc3 is free here: a
            # dedicated wait-free junk chain on it keeps the clock up while
            # the serial projection groups wait on their psum slots)
            for k, (tt, nb) in enumerate(
                    (tt, nb) for tt in range(12, NST) for nb in range(2)):
                proj_group(tt, nb, junk=False,
                           slot=["avc0", "avc1", None, "avc2"][k % 4])
                jt = avpool.tile([P, 512], FP, tag="avc3", bufs=1,
                                 name=f"jt{k}")
                nc.tensor.matmul(jt[:, :512], lhsT=warm[:, :P], rhs=warm[:],
                                 start=True, stop=True)

    nc.compile()
    return nc


def _get_nc():
    if "nc" not in _CACHE:
        _CACHE["nc"] = _build_nc()
    return _CACHE["nc"]


def _in_maps(x, Wq, Wk, Wv, Wp):
    import ml_dtypes

    def f8(a):
        return np.clip(a, -240.0, 240.0).astype(ml_dtypes.float8_e4m3)

    tri = np.tril(np.full((P, P), NEG, np.float32), -1).astype(ml_dtypes.bfloat16)
    eye = np.eye(P, dtype=np.float32).astype(ml_dtypes.bfloat16)
    maps = []
    for b in range(B):
        xT = np.ascontiguousarray(x[b].T)
        for g in range(2):
            heads = range(8 * g, 8 * g + 8)
            # q/k projection weights are pre-scaled by 32 so they sit in the
            # fp8e4m3 sweet spot (std ~1); the exp scale divides the extra
            # 32*32 back out.
            wq = np.ascontiguousarray(np.concatenate([Wq[h] for h in heads], 1))
            wk = np.ascontiguousarray(np.concatenate([Wk[h] for h in heads], 1))
            maps.append({
                "xt": xT.astype(ml_dtypes.bfloat16),
                "xt8": f8(xT),
                "wq8": f8(32.0 * wq),
                "wk8": f8(32.0 * wk),
                "wv": np.ascontiguousarray(np.concatenate([Wv[h] for h in heads], 1)).astype(ml_dtypes.bfloat16),
                "wpt": np.ascontiguousarray(Wp[:, 512 * g:512 * g + 512].T).astype(ml_dtypes.bfloat16),
                "tri": tri,
                "eye": eye,
            })
    return maps


def kernel(x, Wq, Wk, Wv, Wp, bp):
    from concourse.bass_utils import run_bass_kernel_spmd

    x = np.asarray(x, np.float32)
    Wq = np.asarray(Wq, np.float32)
    Wk = np.asarray(Wk, np.float32)
    Wv = np.asarray(Wv, np.float32)
    Wp = np.asarray(Wp, np.float32)
    bp = np.asarray(bp, np.float32)

    nc = _get_nc()
    res = run_bass_kernel_spmd(nc, _in_maps(x, Wq, Wk, Wv, Wp), list(range(8)))
    y = np.empty((B, T, C), np.float32)
    for b in range(B):
        y[b] = (res.results[2 * b]["y"].astype(np.float32)
                + res.results[2 * b + 1]["y"].astype(np.float32) + bp)
    return y

